# revision 4
# baseline (speedup 1.0000x reference)
"""Channel-attention (CAM) Trainium2 kernel.

Reference computation (per batch b of 16):
    q   = x[b].reshape(C, HW)                  # C=512, HW=4096
    sim = q @ q.T                              # [C, C], symmetric
    attn = softmax(max(sim) - sim, axis=-1)    # == exp(min_r - sim) / Z_r
    out[b] = gamma * attn @ q + x[b]

Sharding: data-parallel over batch across 8 NeuronCores (2 batches/core).

Kernel design (per core):
  - All matmuls in float32r (fp32 storage, ~13-bit mantissa, full PE rate at
    N>=256). End-to-end output error vs f32 reference ~1e-4 rel L2.
  - sim is symmetric: compute upper-triangle block rows only (widths 512,
    384, 256, 256) and fill the missing lower blocks by PE-transposing
    upper blocks.
  - softmax via ACT: p = exp(min_r - sim) with accum_out giving Z in the
    same pass; scale rows by gamma/Z (DVE), PE-transpose to build lhsT and
    add identity to the diagonal so the second matmul directly produces
    gamma*attn@q + q = out (residual folded in).
"""
import sys

if "/opt/trn_rl_repo" not in sys.path:
    sys.path.insert(0, "/opt/trn_rl_repo")

import numpy as np

B, C, H, W = 16, 512, 64, 64
HW = H * W
NCORES = 8
NB = B // NCORES          # batches per core
P = 128
CB = C // P               # 4 channel blocks
KN = HW // P              # 32 contraction chunks for sim
NJ = HW // 512            # 8 output column chunks

_BUILD_CACHE = {}


def build_bass():
    import concourse.bacc as bacc
    import concourse.tile as tile
    from concourse import mybir
    from concourse.masks import make_identity

    f32 = mybir.dt.float32
    f32r = mybir.dt.float32r
    AX = mybir.AxisListType
    ALU = mybir.AluOpType
    ACTF = mybir.ActivationFunctionType

    nc = bacc.Bacc()
    x_ext = nc.declare_dram_parameter("x", [NB, C, HW], f32, isOutput=False)
    g_ext = nc.declare_dram_parameter("gamma", [1], f32, isOutput=False)
    o_ext = nc.declare_dram_parameter("out", [NB, C, HW], f32, isOutput=True)

    # alternate PSUM->SBUF copies between ACT and DVE to balance engines
    _flip = [0]

    with tile.TileContext(nc) as tc:
        with (
            tc.tile_pool(name="const", bufs=1) as const,
            tc.tile_pool(name="xchunk", bufs=8) as xchunk,
            tc.tile_pool(name="qr", bufs=4) as qrp,
            tc.tile_pool(name="qt", bufs=33) as qtp,
            tc.tile_pool(name="pp", bufs=4) as pp,
            tc.tile_pool(name="osb", bufs=4) as osb,
            tc.tile_pool(name="tri", bufs=3) as trip,
            tc.tile_pool(name="vec", bufs=10) as vec,
            tc.tile_pool(name="psA", bufs=2, space="PSUM") as psA,
            tc.tile_pool(name="psim", bufs=4, space="PSUM") as psimp,
            tc.tile_pool(name="pfeat", bufs=2, space="PSUM") as pfeat,
        ):
            def copyback(dst, src):
                if _flip[0] % 2 == 0:
                    nc.scalar.copy(dst, src)
                else:
                    nc.vector.tensor_copy(dst, src)
                _flip[0] += 1

            ident_f = const.tile([P, P], f32)
            make_identity(nc, ident_f)
            ident_r = const.tile([P, P], f32r)
            nc.vector.tensor_copy(ident_r[:], ident_f[:])
            gamma_sb = const.tile([P, 1], f32)
            nc.sync.dma_start(out=gamma_sb[:], in_=g_ext[:].to_broadcast([P, 1]))

            for b in range(NB):
                # ---- load x chunks, round to f32r ----
                qr_t = [qrp.tile([P, HW], f32r, tag="qr", name=f"qr{b}_{i}") for i in range(CB)]
                for mi in range(CB):
                    for j in range(NJ):
                        xt = xchunk.tile([P, 512], f32, tag="xc")
                        nc.sync.dma_start(
                            out=xt[:],
                            in_=x_ext[b, mi * P:(mi + 1) * P, j * 512:(j + 1) * 512],
                        )
                        nc.vector.tensor_copy(
                            qr_t[mi][:, j * 512:(j + 1) * 512], xt[:]
                        )

                # ---- build qT [n, c] tiles via PE transpose ----
                qt_t = [qtp.tile([P, C], f32r, tag="qt", name=f"qt{b}_{i}") for i in range(KN)]
                for kn in range(KN):
                    pst = psA.tile([P, C], f32r, tag="psA")
                    for ci in range(CB):
                        nc.tensor.transpose(
                            pst[:, ci * P:(ci + 1) * P],
                            qr_t[ci][:, kn * P:(kn + 1) * P],
                            ident_r[:],
                        )
                    copyback(qt_t[kn][:], pst[:])

                # ---- sim = q @ q.T, upper-triangle block rows ----
                psim = []
                for mi in range(CB):
                    c0 = min(mi * P, 2 * P)   # 0, 128, 256, 256
                    pt = psimp.tile([P, C], f32, tag="psim")
                    psim.append(pt)
                    for kn in range(KN):
                        nc.tensor.matmul(
                            pt[:, c0:],
                            qt_t[kn][:, mi * P:(mi + 1) * P],
                            qt_t[kn][:, c0:],
                            start=(kn == 0),
                            stop=(kn == KN - 1),
                        )
                # fill lower blocks by symmetry: dst block (i, j) = T(src (j, i))
                for (i, j) in [(1, 0), (2, 0), (2, 1), (3, 0), (3, 1)]:
                    tmp = trip.tile([P, P], f32, tag="tri")
                    nc.scalar.copy(tmp[:], psim[j][:, i * P:(i + 1) * P])
                    nc.tensor.transpose(
                        psim[i][:, j * P:(j + 1) * P], tmp[:], ident_f[:]
                    )

                # ---- softmax rows: p_s = (gamma/Z) * exp(min_r - sim) ----
                ps_t = []
                for mi in range(CB):
                    mrow = vec.tile([P, 1], f32, tag="mrow")
                    nc.vector.tensor_reduce(
                        mrow[:], psim[mi][:], axis=AX.X, op=ALU.min
                    )
                    zrow = vec.tile([P, 1], f32, tag="zrow")
                    p_t = pp.tile([P, C], f32r, tag="p")
                    nc.scalar.activation(
                        p_t[:], psim[mi][:], ACTF.Exp,
                        bias=mrow[:], scale=-1.0, accum_out=zrow[:],
                    )
                    rz = vec.tile([P, 1], f32, tag="rz")
                    nc.vector.reciprocal(rz[:], zrow[:])
                    rzg = vec.tile([P, 1], f32, tag="rzg")
                    nc.vector.tensor_mul(rzg[:], rz[:], gamma_sb[:])
                    p_s = pp.tile([P, C], f32r, tag="psc")
                    nc.vector.tensor_scalar_mul(p_s[:], p_t[:], rzg[:])
                    ps_t.append(p_s)

                # ---- lhsT for mm2: pT = T(p_s) + I ----
                pt_t = []
                for kd in range(CB):
                    pst = psA.tile([P, C], f32r, tag="psA")
                    for ci in range(CB):
                        nc.tensor.transpose(
                            pst[:, ci * P:(ci + 1) * P],
                            ps_t[ci][:, kd * P:(kd + 1) * P],
                            ident_r[:],
                        )
                    t = pp.tile([P, C], f32r, tag="pt")
                    copyback(t[:], pst[:])
                    nc.vector.tensor_add(
                        t[:, kd * P:(kd + 1) * P],
                        t[:, kd * P:(kd + 1) * P],
                        ident_r[:],
                    )
                    pt_t.append(t)

                # ---- out = (gamma*diag(1/Z)*P + I) @ q ----
                for mi in range(CB):
                    for nj in range(NJ):
                        pf = pfeat.tile([P, 512], f32, tag="pf")
                        for kd in range(CB):
                            nc.tensor.matmul(
                                pf[:],
                                pt_t[kd][:, mi * P:(mi + 1) * P],
                                qr_t[kd][:, nj * 512:(nj + 1) * 512],
                                start=(kd == 0),
                                stop=(kd == CB - 1),
                            )
                        ot = osb.tile([P, 512], f32, tag="ot")
                        copyback(ot[:], pf[:])
                        nc.sync.dma_start(
                            out=o_ext[b, mi * P:(mi + 1) * P, nj * 512:(nj + 1) * 512],
                            in_=ot[:],
                        )

    nc.finalize()
    return nc


def get_bass():
    if "nc" not in _BUILD_CACHE:
        _BUILD_CACHE["nc"] = build_bass()
    return _BUILD_CACHE["nc"]


def make_in_maps(x, gamma):
    x = np.ascontiguousarray(np.asarray(x, dtype=np.float32)).reshape(B, C, HW)
    gamma = np.asarray(gamma, dtype=np.float32).reshape(1)
    return [
        {"x": x[i * NB:(i + 1) * NB], "gamma": gamma}
        for i in range(NCORES)
    ]


def run(x, gamma, trace=False, **trace_kwargs):
    from concourse.bass_utils import run_bass_kernel_spmd

    nc = get_bass()
    res = run_bass_kernel_spmd(
        nc, make_in_maps(x, gamma), core_ids=list(range(NCORES)),
        trace=trace, **trace_kwargs,
    )
    out = np.concatenate([res.results[i]["out"] for i in range(NCORES)], axis=0)
    return out.reshape(B, C, H, W), res


def kernel(x, gamma):
    out, _ = run(x, gamma, trace=False)
    return out


# revision 6
# speedup vs baseline: 1.0025x; 1.0025x over previous
"""Channel-attention (CAM) Trainium2 kernel.

Reference computation (per batch b of 16):
    q   = x[b].reshape(C, HW)                  # C=512, HW=4096
    sim = q @ q.T                              # [C, C], symmetric
    attn = softmax(max(sim) - sim, axis=-1)    # == exp(min_r - sim) / Z_r
    out[b] = gamma * attn @ q + x[b]

Sharding: data-parallel over batch across 8 NeuronCores (2 batches/core).

Kernel design (per core):
  - All matmuls in float32r (fp32 storage, ~13-bit mantissa, full PE rate at
    N>=256). End-to-end output error vs f32 reference ~1e-4 rel L2.
  - sim is symmetric: compute upper-triangle block rows only (widths 512,
    384, 256, 256) and fill the missing lower blocks by PE-transposing
    upper blocks.
  - softmax via ACT: p = exp(min_r - sim) with accum_out giving Z in the
    same pass; scale rows by gamma/Z (DVE), PE-transpose to build lhsT and
    add identity to the diagonal so the second matmul directly produces
    gamma*attn@q + q = out (residual folded in).
"""
import sys

if "/opt/trn_rl_repo" not in sys.path:
    sys.path.insert(0, "/opt/trn_rl_repo")

import numpy as np

B, C, H, W = 16, 512, 64, 64
HW = H * W
NCORES = 8
NB = B // NCORES          # batches per core
P = 128
CB = C // P               # 4 channel blocks
KN = HW // P              # 32 contraction chunks for sim
NJ = HW // 512            # 8 output column chunks

_BUILD_CACHE = {}


def build_bass():
    import concourse.bacc as bacc
    import concourse.tile as tile
    from concourse import mybir
    from concourse.masks import make_identity

    f32 = mybir.dt.float32
    f32r = mybir.dt.float32r
    AX = mybir.AxisListType
    ALU = mybir.AluOpType
    ACTF = mybir.ActivationFunctionType

    nc = bacc.Bacc()
    x_ext = nc.declare_dram_parameter("x", [NB, C, HW], f32, isOutput=False)
    g_ext = nc.declare_dram_parameter("gamma", [1], f32, isOutput=False)
    o_ext = nc.declare_dram_parameter("out", [NB, C, HW], f32, isOutput=True)

    # alternate PSUM->SBUF copies between ACT and DVE to balance engines
    _flip = [0]

    with tile.TileContext(nc) as tc:
        with (
            tc.tile_pool(name="const", bufs=1) as const,
            tc.tile_pool(name="xchunk", bufs=3) as xchunk,
            tc.tile_pool(name="qr", bufs=5) as qrp,
            tc.tile_pool(name="qt", bufs=33) as qtp,
            tc.tile_pool(name="pp", bufs=4) as pp,
            tc.tile_pool(name="osb", bufs=2) as osb,
            tc.tile_pool(name="tri", bufs=3) as trip,
            tc.tile_pool(name="vec", bufs=10) as vec,
            tc.tile_pool(name="psA", bufs=2, space="PSUM") as psA,
            tc.tile_pool(name="psim", bufs=4, space="PSUM") as psimp,
            tc.tile_pool(name="pfeat", bufs=2, space="PSUM") as pfeat,
        ):
            def copyback(dst, src):
                if _flip[0] % 2 == 0:
                    nc.scalar.copy(dst, src)
                else:
                    nc.vector.tensor_copy(dst, src)
                _flip[0] += 1

            ident_f = const.tile([P, P], f32)
            make_identity(nc, ident_f)
            ident_r = const.tile([P, P], f32r)
            nc.vector.tensor_copy(ident_r[:], ident_f[:])
            gamma_sb = const.tile([P, 1], f32)
            nc.sync.dma_start(out=gamma_sb[:], in_=g_ext[:].to_broadcast([P, 1]))

            NW = 4          # column waves per batch
            WC = HW // NW   # 1024 columns per wave
            KQ = WC // P    # 8 transpose chunks per wave

            for b in range(NB):
                qr_t = [qrp.tile([P, HW], f32r, tag="qr", name=f"qr{b}_{i}")
                        for i in range(CB)]
                qt_t = [qtp.tile([P, C], f32r, tag="qt", name=f"qt{b}_{i}")
                        for i in range(KN)]
                psim = [psimp.tile([P, C], f32, tag="psim", name=f"psim{b}_{i}")
                        for i in range(CB)]

                c0s = [min(mi * P, 2 * P) for mi in range(CB)]  # 0,128,256,256

                for w in range(NW):
                    # ---- load + round this wave's columns ----
                    for mi in range(CB):
                        xt = xchunk.tile([P, WC], f32, tag="xc")
                        nc.sync.dma_start(
                            out=xt[:],
                            in_=x_ext[b, mi * P:(mi + 1) * P, w * WC:(w + 1) * WC],
                        )
                        nc.vector.tensor_copy(
                            qr_t[mi][:, w * WC:(w + 1) * WC], xt[:]
                        )
                    # ---- qT transposes + sim matmuls for this wave ----
                    for kq in range(KQ):
                        kn = w * KQ + kq
                        pst = psA.tile([P, C], f32r, tag="psA")
                        for ci in range(CB):
                            nc.tensor.transpose(
                                pst[:, ci * P:(ci + 1) * P],
                                qr_t[ci][:, kn * P:(kn + 1) * P],
                                ident_r[:],
                            )
                        copyback(qt_t[kn][:], pst[:])
                        for mi in range(CB):
                            c0 = c0s[mi]
                            nc.tensor.matmul(
                                psim[mi][:, c0:],
                                qt_t[kn][:, mi * P:(mi + 1) * P],
                                qt_t[kn][:, c0:],
                                start=(kn == 0),
                                stop=(kn == KN - 1),
                            )

                # ---- fill lower blocks by symmetry: (i, j) = T((j, i)) ----
                for (i, j) in [(1, 0), (2, 0), (2, 1), (3, 0), (3, 1)]:
                    tmp = trip.tile([P, P], f32, tag="tri")
                    nc.scalar.copy(tmp[:], psim[j][:, i * P:(i + 1) * P])
                    nc.tensor.transpose(
                        psim[i][:, j * P:(j + 1) * P], tmp[:], ident_f[:]
                    )

                # ---- softmax rows: p_s = (gamma/Z) * exp(min_r - sim) ----
                ps_t = []
                for mi in range(CB):
                    mrow = vec.tile([P, 1], f32, tag="mrow")
                    nc.vector.tensor_reduce(
                        mrow[:], psim[mi][:], axis=AX.X, op=ALU.min
                    )
                    zrow = vec.tile([P, 1], f32, tag="zrow")
                    p_t = pp.tile([P, C], f32r, tag="p")
                    nc.scalar.activation(
                        p_t[:], psim[mi][:], ACTF.Exp,
                        bias=mrow[:], scale=-1.0, accum_out=zrow[:],
                    )
                    rz = vec.tile([P, 1], f32, tag="rz")
                    nc.vector.reciprocal(rz[:], zrow[:])
                    rzg = vec.tile([P, 1], f32, tag="rzg")
                    nc.vector.tensor_mul(rzg[:], rz[:], gamma_sb[:])
                    p_s = pp.tile([P, C], f32r, tag="psc")
                    nc.vector.tensor_scalar_mul(p_s[:], p_t[:], rzg[:])
                    ps_t.append(p_s)

                # ---- lhsT for mm2: pT = T(p_s) + I ----
                pt_t = []
                for kd in range(CB):
                    pst = psA.tile([P, C], f32r, tag="psA")
                    for ci in range(CB):
                        nc.tensor.transpose(
                            pst[:, ci * P:(ci + 1) * P],
                            ps_t[ci][:, kd * P:(kd + 1) * P],
                            ident_r[:],
                        )
                    t = pp.tile([P, C], f32r, tag="pt", bufs=4)
                    copyback(t[:], pst[:])
                    nc.vector.tensor_add(
                        t[:, kd * P:(kd + 1) * P],
                        t[:, kd * P:(kd + 1) * P],
                        ident_r[:],
                    )
                    pt_t.append(t)

                # ---- out = (gamma*diag(1/Z)*P + I) @ q, staged stores ----
                for mi in range(CB):
                    for half in range(2):
                        stg = osb.tile([P, HW // 2], f32, tag="ot")
                        for njh in range(NJ // 2):
                            nj = half * (NJ // 2) + njh
                            pf = pfeat.tile([P, 512], f32, tag="pf")
                            for kd in range(CB):
                                nc.tensor.matmul(
                                    pf[:],
                                    pt_t[kd][:, mi * P:(mi + 1) * P],
                                    qr_t[kd][:, nj * 512:(nj + 1) * 512],
                                    start=(kd == 0),
                                    stop=(kd == CB - 1),
                                )
                            copyback(stg[:, njh * 512:(njh + 1) * 512], pf[:])
                        nc.sync.dma_start(
                            out=o_ext[b, mi * P:(mi + 1) * P,
                                      half * (HW // 2):(half + 1) * (HW // 2)],
                            in_=stg[:],
                        )

    nc.finalize()
    return nc


def get_bass():
    if "nc" not in _BUILD_CACHE:
        _BUILD_CACHE["nc"] = build_bass()
    return _BUILD_CACHE["nc"]


def make_in_maps(x, gamma):
    x = np.ascontiguousarray(np.asarray(x, dtype=np.float32)).reshape(B, C, HW)
    gamma = np.asarray(gamma, dtype=np.float32).reshape(1)
    return [
        {"x": x[i * NB:(i + 1) * NB], "gamma": gamma}
        for i in range(NCORES)
    ]


def run(x, gamma, trace=False, **trace_kwargs):
    from concourse.bass_utils import run_bass_kernel_spmd

    nc = get_bass()
    res = run_bass_kernel_spmd(
        nc, make_in_maps(x, gamma), core_ids=list(range(NCORES)),
        trace=trace, **trace_kwargs,
    )
    out = np.concatenate([res.results[i]["out"] for i in range(NCORES)], axis=0)
    return out.reshape(B, C, H, W), res


def kernel(x, gamma):
    out, _ = run(x, gamma, trace=False)
    return out


# revision 10
# speedup vs baseline: 1.1468x; 1.1439x over previous
"""Channel-attention (CAM) Trainium2 kernel.

Reference computation (per batch b of 16):
    q   = x[b].reshape(C, HW)                  # C=512, HW=4096
    sim = q @ q.T                              # [C, C], symmetric
    attn = softmax(max(sim) - sim, axis=-1)    # == exp(min_r - sim) / Z_r
    out[b] = gamma * attn @ q + x[b]

Sharding: data-parallel over batch across 8 NeuronCores (2 batches/core).

Kernel design (per core):
  - All matmuls in float32r (fp32 storage, ~13-bit mantissa, full PE rate at
    N>=256). End-to-end output error vs f32 reference ~1e-4 rel L2.
  - sim is symmetric: compute upper-triangle block rows only (widths 512,
    384, 256, 256) and fill the missing lower blocks by PE-transposing
    upper blocks.
  - softmax via ACT: p = exp(min_r - sim) with accum_out giving Z in the
    same pass; scale rows by gamma/Z (DVE), PE-transpose to build lhsT and
    add identity to the diagonal so the second matmul directly produces
    gamma*attn@q + q = out (residual folded in).
"""
import sys

if "/opt/trn_rl_repo" not in sys.path:
    sys.path.insert(0, "/opt/trn_rl_repo")

import numpy as np

B, C, H, W = 16, 512, 64, 64
HW = H * W
NCORES = 8
NB = B // NCORES          # batches per core
P = 128
CB = C // P               # 4 channel blocks
KN = HW // P              # 32 contraction chunks for sim
NJ = HW // 512            # 8 output column chunks

_BUILD_CACHE = {}


def build_bass():
    import concourse.bacc as bacc
    import concourse.tile as tile
    from concourse import mybir
    from concourse.masks import make_identity

    f32 = mybir.dt.float32
    f32r = mybir.dt.float32r
    AX = mybir.AxisListType
    ALU = mybir.AluOpType
    ACTF = mybir.ActivationFunctionType

    nc = bacc.Bacc()
    x_ext = nc.declare_dram_parameter("x", [NB, C, HW], f32, isOutput=False)
    g_ext = nc.declare_dram_parameter("gamma", [1], f32, isOutput=False)
    o_ext = nc.declare_dram_parameter("out", [NB, C, HW], f32, isOutput=True)

    # alternate PSUM->SBUF copies between ACT and DVE to balance engines
    _flip = [0]

    with tile.TileContext(nc) as tc:
        with (
            tc.tile_pool(name="const", bufs=1) as const,
            tc.tile_pool(name="xchunk", bufs=4) as xchunk,
            tc.tile_pool(name="qr", bufs=8) as qrp,
            tc.tile_pool(name="qt", bufs=11) as qtp,
            tc.tile_pool(name="pp", bufs=4) as pp,
            tc.tile_pool(name="osb", bufs=2) as osb,
            tc.tile_pool(name="tri", bufs=3) as trip,
            tc.tile_pool(name="vec", bufs=6) as vec,
            tc.tile_pool(name="psA", bufs=2, space="PSUM") as psA,
            tc.tile_pool(name="psim", bufs=4, space="PSUM") as psimp,
            tc.tile_pool(name="pfeat", bufs=2, space="PSUM") as pfeat,
        ):
            def copyback(dst, src):
                if _flip[0] % 2 == 0:
                    nc.scalar.copy(dst, src)
                else:
                    nc.vector.tensor_copy(dst, src)
                _flip[0] += 1

            ident_f = const.tile([P, P], f32)
            make_identity(nc, ident_f)
            ident_r = const.tile([P, P], f32r)
            nc.vector.tensor_copy(ident_r[:], ident_f[:])
            gamma_sb = const.tile([P, 1], f32)
            nc.sync.dma_start(out=gamma_sb[:], in_=g_ext[:].to_broadcast([P, 1]))

            NW = 4          # column waves per batch
            WC = HW // NW   # 1024 columns per wave
            KQ = WC // P    # 8 transpose chunks per wave

            for b in range(NB):
                qr_t = [qrp.tile([P, HW], f32r, tag="qr", name=f"qr{b}_{i}")
                        for i in range(CB)]
                psim = [psimp.tile([P, C], f32, tag="psim", name=f"psim{b}_{i}")
                        for i in range(CB)]

                c0s = [min(mi * P, 2 * P) for mi in range(CB)]  # 0,128,256,256

                # phase 1+2 pipeline: per wave, load+round columns, transpose
                # to qT, and run sim matmuls one kn behind the transposes.
                qt_tiles = {}

                def mm1(kn):
                    for mi in range(CB):
                        c0 = c0s[mi]
                        nc.tensor.matmul(
                            psim[mi][:, c0:],
                            qt_tiles[kn][:, mi * P:(mi + 1) * P],
                            qt_tiles[kn][:, c0:],
                            start=(kn == 0),
                            stop=(kn == KN - 1),
                        )

                prev_kn = None
                for w in range(NW):
                    for mi in range(CB):
                        xt = xchunk.tile([P, WC], f32, tag="xc")
                        nc.sync.dma_start(
                            out=xt[:],
                            in_=x_ext[b, mi * P:(mi + 1) * P, w * WC:(w + 1) * WC],
                        )
                        nc.vector.tensor_copy(
                            qr_t[mi][:, w * WC:(w + 1) * WC], xt[:]
                        )
                    for kq in range(KQ):
                        kn = w * KQ + kq
                        pst = psA.tile([P, C], f32r, tag="psA")
                        for ci in range(CB):
                            nc.tensor.transpose(
                                pst[:, ci * P:(ci + 1) * P],
                                qr_t[ci][:, kn * P:(kn + 1) * P],
                                ident_r[:],
                            )
                        qt = qtp.tile([P, C], f32r, tag="qt", name=f"qt{b}_{kn}")
                        qt_tiles[kn] = qt
                        copyback(qt[:], pst[:])
                        if prev_kn is not None:
                            mm1(prev_kn)
                        prev_kn = kn
                mm1(prev_kn)

                # ---- fill lower blocks by symmetry: (i, j) = T((j, i)) ----
                for (i, j) in [(1, 0), (2, 0), (2, 1), (3, 0), (3, 1)]:
                    tmp = trip.tile([P, P], f32, tag="tri")
                    nc.scalar.copy(tmp[:], psim[j][:, i * P:(i + 1) * P])
                    nc.tensor.transpose(
                        psim[i][:, j * P:(j + 1) * P], tmp[:], ident_f[:]
                    )

                # ---- softmax rows: p_s = (gamma/Z) * exp(min_r - sim) ----
                ps_t = []
                for mi in range(CB):
                    mrow = vec.tile([P, 1], f32, tag="mrow")
                    nc.vector.tensor_reduce(
                        mrow[:], psim[mi][:], axis=AX.X, op=ALU.min
                    )
                    zrow = vec.tile([P, 1], f32, tag="zrow")
                    p_t = pp.tile([P, C], f32r, tag="p", bufs=3)
                    nc.scalar.activation(
                        p_t[:], psim[mi][:], ACTF.Exp,
                        bias=mrow[:], scale=-1.0, accum_out=zrow[:],
                    )
                    rz = vec.tile([P, 1], f32, tag="rz")
                    nc.vector.reciprocal(rz[:], zrow[:])
                    rzg = vec.tile([P, 1], f32, tag="rzg")
                    nc.vector.tensor_mul(rzg[:], rz[:], gamma_sb[:])
                    p_s = pp.tile([P, C], f32r, tag="psc", bufs=4)
                    nc.vector.tensor_scalar_mul(p_s[:], p_t[:], rzg[:])
                    ps_t.append(p_s)

                # ---- lhsT for mm2: pT = T(p_s) + I ----
                pt_t = []
                for kd in range(CB):
                    pst = psA.tile([P, C], f32r, tag="psA")
                    for ci in range(CB):
                        nc.tensor.transpose(
                            pst[:, ci * P:(ci + 1) * P],
                            ps_t[ci][:, kd * P:(kd + 1) * P],
                            ident_r[:],
                        )
                    t = pp.tile([P, C], f32r, tag="pt")
                    copyback(t[:], pst[:])
                    nc.vector.tensor_add(
                        t[:, kd * P:(kd + 1) * P],
                        t[:, kd * P:(kd + 1) * P],
                        ident_r[:],
                    )
                    pt_t.append(t)

                # ---- out = (gamma*diag(1/Z)*P + I) @ q, staged stores ----
                for mi in range(CB):
                    for half in range(2):
                        stg = osb.tile([P, HW // 2], f32, tag="ot")
                        for njh in range(NJ // 2):
                            nj = half * (NJ // 2) + njh
                            pf = pfeat.tile([P, 512], f32, tag="pf")
                            for kd in range(CB):
                                nc.tensor.matmul(
                                    pf[:],
                                    pt_t[kd][:, mi * P:(mi + 1) * P],
                                    qr_t[kd][:, nj * 512:(nj + 1) * 512],
                                    start=(kd == 0),
                                    stop=(kd == CB - 1),
                                )
                            copyback(stg[:, njh * 512:(njh + 1) * 512], pf[:])
                        nc.scalar.dma_start(
                            out=o_ext[b, mi * P:(mi + 1) * P,
                                      half * (HW // 2):(half + 1) * (HW // 2)],
                            in_=stg[:],
                        )

    nc.finalize()
    return nc


def get_bass():
    if "nc" not in _BUILD_CACHE:
        _BUILD_CACHE["nc"] = build_bass()
    return _BUILD_CACHE["nc"]


def make_in_maps(x, gamma):
    x = np.ascontiguousarray(np.asarray(x, dtype=np.float32)).reshape(B, C, HW)
    gamma = np.asarray(gamma, dtype=np.float32).reshape(1)
    return [
        {"x": x[i * NB:(i + 1) * NB], "gamma": gamma}
        for i in range(NCORES)
    ]


def run(x, gamma, trace=False, **trace_kwargs):
    from concourse.bass_utils import run_bass_kernel_spmd

    nc = get_bass()
    res = run_bass_kernel_spmd(
        nc, make_in_maps(x, gamma), core_ids=list(range(NCORES)),
        trace=trace, **trace_kwargs,
    )
    out = np.concatenate([res.results[i]["out"] for i in range(NCORES)], axis=0)
    return out.reshape(B, C, H, W), res


def kernel(x, gamma):
    out, _ = run(x, gamma, trace=False)
    return out
